# revision 18
# baseline (speedup 1.0000x reference)
"""Causal self-attention Trainium2 kernel (V2).

Problem: B=8, T=1024, C=2048, 16 heads x 128 head-dim, fp32.
Sharding: data-parallel over batch -- each of the 8 NeuronCores computes one
batch element end-to-end; no collectives.

V2 dataflow (everything resident in SBUF, no DRAM spills):
  phase A+B (pipelined per row-tile): x -> xT (PE transpose, f32r),
    v = x @ W_v + b_v into per-head SBUF tiles vh[h] (bf16), laid out
    [k-part, d] per 128-k-block with a ones column appended per block.
  phase C per head (software-pipelined, iteration h emits):
    qk(h):   qT/kT[d,T] bf16 <- ACT(bias) <- PE(W_qk^T @ xT)
    sc(h):   sT[k,q]    <- kT-block^T-free @ qT 256-chunk; exps = Exp (ACT,
             bf16), causal-masked (DVE) on diagonal blocks
    av(h-1): yacc2[q,130] (PSUM) += exps-128-slice^T-free @ [v|1] (moving 129)
             -> col 128 = softmax denominator; rden = 1/den (DVE);
             y_norm[q,d] = yacc2 * rden (DVE tensor_scalar, per-partition);
             yT via PE transpose (bf16) -> yTh[h] resident SBUF
  phase D: out = y @ W_proj + b (hh-outer over 512-wide n-chunks, 8 PSUM
    banks hold [T, 512]; lhsT = yTh slices (bf16), rhs = wp (f32r), so the
    first chunk can start as soon as wp[0] is DMA'd; no spill reload).
"""

import math
from contextlib import ExitStack

import numpy as np

import concourse.bass as bass
import concourse.mybir as mybir
import concourse.tile as tile
from concourse.masks import make_identity
from concourse.vector_clock import ScopedClock

F32 = mybir.dt.float32
F32R = mybir.dt.float32r
BF16 = mybir.dt.bfloat16
F8E4 = mybir.dt.float8e4

B, T, C = 8, 1024, 2048
NH, HD = 16, 128
P = 128
TT = T // P            # 8 row tiles
CT = C // P            # 16 channel tiles
QP = 256               # q-pair width for score matmuls
NQP = T // QP          # 4 q-pairs
VW = 130               # per-k-block stride in vh tiles: 128 v cols + ones col
SM_SCALE = 1.0 / math.sqrt(HD)

N_CORES = 8

# --------------------------------------------------------------------------
# Walrus workaround: this container's walrus rejects any instruction with
# more than one sync wait command. Split multi-wait instructions into a
# chain of single-wait NoOps/Drains on the same engine (engine queues
# process waits in order, so semantics are unchanged).
# --------------------------------------------------------------------------
_orig_commit_instruction = tile.TileContext._commit_instruction


def _patched_commit_instruction(self, inst, lazy_reg_writes=True):
    si = inst.sync_info
    if (
        si is not None
        and len(si.on_wait) > 1
        and inst.engine != mybir.EngineType.Unassigned
    ):
        waits = list(si.on_wait)
        for w in waits[:-1]:
            nop = mybir.InstNoOp(
                name=self.nc.get_next_instruction_name(),
                engine=inst.engine,
                bass_nofuse=True,
                sync_info=mybir.SyncInfo(on_wait=[w], on_update=[]),
            )
            _orig_commit_instruction(self, nop, lazy_reg_writes=False)
        inst.sync_info = mybir.SyncInfo(
            on_wait=[waits[-1]], on_update=list(si.on_update)
        )
    return _orig_commit_instruction(self, inst, lazy_reg_writes=lazy_reg_writes)


def _patched_drain_and_barrier(self, tick_clock, wait_clock):
    drain_inst = self.nc.sync.drain()
    wait_clock.add_sem_waits(
        drain_inst.ins, ScopedClock({None: tick_clock.global_clock})
    )
    si = drain_inst.ins.sync_info
    if si is not None and len(si.on_wait) > 1:
        waits = list(si.on_wait)
        drain_inst.ins.sync_info = mybir.SyncInfo(
            on_wait=[waits[0]], on_update=list(si.on_update)
        )
        for w in waits[1:]:
            d2 = self.nc.sync.drain()
            d2.ins.sync_info = mybir.SyncInfo(on_wait=[w], on_update=[])
    self.nc.all_engine_barrier()
    assert self.sems is not None
    popped = self.nc._tile_sem_poison_stack.pop()
    assert popped is self._sem_poison
    self.nc.clear_and_free_semaphores(list(self.sems.allocated().values()))
    self.nc.all_engine_barrier()


def _apply_patches():
    tile.TileContext._commit_instruction = _patched_commit_instruction
    tile.TileContext._drain_and_barrier = _patched_drain_and_barrier


# --------------------------------------------------------------------------
# Kernel builder
# --------------------------------------------------------------------------

def build_kernel(mode: str = "f32r") -> bass.Bass:
    """mode: 'f32r' (fast path: f32r projections, bf16 attention) or
    'f32' (full fp32 matmuls, slower; debugging)."""
    _apply_patches()
    mm_dt = F32R if mode == "f32r" else F32
    at_dt = BF16 if mode == "f32r" else F32
    a8_dt = F8E4 if mode == "f32r" else F32

    nc = bass.Bass("TRN2", target_bir_lowering=False, debug=False)

    x_ap = nc.dram_tensor("x", [T, C], F32, kind="ExternalInput").ap()
    wa_ap = nc.dram_tensor("W_attn", [C, 3 * C], F32, kind="ExternalInput").ap()
    ba_ap = nc.dram_tensor("b_attn", [3 * C], F32, kind="ExternalInput").ap()
    wp_ap = nc.dram_tensor("W_proj", [C, C], F32, kind="ExternalInput").ap()
    bp_ap = nc.dram_tensor("b_proj", [C], F32, kind="ExternalInput").ap()
    out_ap = nc.dram_tensor("out", [T, C], F32, kind="ExternalOutput").ap()

    def r(ap):
        return ap.bitcast(mm_dt) if mm_dt is F32R else ap

    # DRAM views
    x_rows = x_ap.rearrange("(i p) c -> i p c", p=P)          # [TT, P, C]
    out_rows = out_ap.rearrange("(i p) c -> i p c", p=P)      # [TT, P, C]
    wa_3d = wa_ap.rearrange("(j p) n -> p j n", p=P)          # [P, CT, 3C]
    wp_rows = wp_ap.rearrange("(h p) n -> h p n", p=P)        # [NH, P, C]
    ba_col = ba_ap.rearrange("(n p one) -> n p one", p=P, one=1)  # [48, P, 1]
    bv_row = ba_ap.rearrange("(n c) -> n c", n=3)             # [3, C]
    bp_row = bp_ap.rearrange("(one c) -> one c", one=1)       # [1, C]

    with tile.TileContext(nc) as tc, ExitStack() as ctx:
        # ---------------- constants ----------------
        const = ctx.enter_context(tc.tile_pool(name="const", bufs=1))
        ident = const.tile([P, P], mm_dt)
        make_identity(nc, ident[:])
        ident_b = const.tile([P, P], at_dt)
        make_identity(nc, ident_b[:])
        # lower-triangular causal mask for diagonal k-blocks:
        # maskT[k, q] = 1 if q >= k else 0 (both diagonal cases reduce to it)
        maskT = const.tile([P, P], a8_dt)
        nc.gpsimd.memset(maskT[:], 1.0)
        nc.gpsimd.affine_select(
            out=maskT[:], in_=maskT[:], compare_op=mybir.AluOpType.is_ge,
            fill=0.0, base=0, pattern=[[1, P]], channel_multiplier=-1)
        # ones row (K=1 bias matmul lhsT)
        ones_row_f = const.tile([1, P], F32)
        nc.vector.memset(ones_row_f[:], 1.0)
        ones_row = const.tile([1, P], mm_dt)
        nc.vector.tensor_copy(ones_row[:], ones_row_f[:])

        # ---------------- resident tensors ----------------
        # yTh spans phases C-D; xT/vh close after attention (work_ctx).
        res_pool = ctx.enter_context(tc.tile_pool(name="resident", bufs=1))
        yTh = [res_pool.tile([P, T], at_dt, tag=f"yTh{h}", name=f"yTh{h}")
               for h in range(NH)]
        # prefetch targets for phase D (W_proj head tiles + bias); the pool
        # must outlive work_ctx, so it is opened here. DMAs for wp_pre are
        # emitted late (during the last attention head).
        pre_pool = ctx.enter_context(tc.tile_pool(name="pre", bufs=1))
        bp_sb = pre_pool.tile([1, C], mm_dt, tag="bp")
        nc.sync.dma_start(bp_sb[:], r(bp_row[:, :]))
        N_PRE = 5
        wp_pre = [pre_pool.tile([P, C], mm_dt, tag=f"wpp{hh}", name=f"wpp{hh}")
                  for hh in range(N_PRE)]
        work_ctx = ExitStack()
        work = work_ctx.enter_context(tc.tile_pool(name="work", bufs=1))
        xT = [work.tile([P, T], at_dt, tag=f"xT{j}", name=f"xT{j}")
              for j in range(CT)]
        # vh[h]: per k-block j, cols [j*VW, j*VW+128) = v rows of block j for
        # head h; col j*VW+128 = 1.0 (softmax denominator); col +129 unused.
        vh = [work.tile([P, TT * VW], a8_dt, tag=f"vh{h}", name=f"vh{h}")
              for h in range(NH)]
        for h in range(NH):
            # split memsets across Pool and DVE so neither gates the start
            if h % 2 == 0:
                nc.gpsimd.memset(vh[h][:], 1.0)
            else:
                nc.vector.memset(vh[h][:], 1.0)

        # ---------------- phase A: x -> xT --------------------------------
        NW = 4                           # n-chunks of W_v (512 wide each)
        CW = C // NW
        with tc.tile_pool(name="psA", bufs=1, space="PSUM") as psA, \
             tc.tile_pool(name="phA", bufs=1) as phA:
            for i in range(TT):
                xa = phA.tile([P, C], mm_dt, tag="xa", bufs=4,
                              name=f"xa{i}")
                nc.sync.dma_start(xa[:], r(x_rows[i]))
                for j in range(CT):
                    tp = psA.tile([P, P], mm_dt, tag="tp", bufs=4)
                    nc.tensor.transpose(tp[:], xa[:, j * P:(j + 1) * P],
                                        ident[:])
                    # rotate evacuation across ACT/DVE/Pool (f32 -> bf16)
                    dst = xT[j][:, i * P:(i + 1) * P]
                    if j % 4 == 0:
                        with nc.allow_low_precision(
                                reason="bf16 x is within budget"):
                            nc.vector.tensor_copy(dst, tp[:].bitcast(F32))
                    elif j % 4 == 2:
                        with nc.allow_low_precision(
                                reason="bf16 x is within budget"):
                            nc.gpsimd.tensor_copy(dst, tp[:].bitcast(F32))
                    else:
                        nc.scalar.activation(
                            dst, tp[:].bitcast(F32),
                            mybir.ActivationFunctionType.Copy)

        # ---------------- phase B: v = x @ W_v + b_v ----------------------
        with tc.tile_pool(name="psA2", bufs=1, space="PSUM") as psA, \
             tc.tile_pool(name="phB", bufs=1) as phB:
            bv_sb = phB.tile([1, C], mm_dt, tag="bv")
            nc.sync.dma_start(bv_sb[:], r(bv_row[2:3, :]))
            wv = {}
            for np_i in range(NW):
                for c in range(CT):
                    wvc = phB.tile([P, CW], mm_dt, tag=f"wv{c}", bufs=1,
                                   name=f"wv{c}_{np_i}")
                    nc.sync.dma_start(
                        wvc[:],
                        r(wa_3d[:, c,
                                2 * C + np_i * CW: 2 * C + (np_i + 1) * CW]))
                    wv[(np_i, c)] = wvc
            for np_i in range(NW):
                for i in range(TT):
                    pv = psA.tile([P, CW], F32, tag="pv", bufs=4,
                                  name=f"pv{np_i}_{i}")
                    for c in range(CT):
                        nc.tensor.matmul(
                            pv[:], xT[c][:, i * P:(i + 1) * P],
                            wv[(np_i, c)][:],
                            start=(c == 0), stop=False)
                    nc.tensor.matmul(
                        pv[:], ones_row[:],
                        bv_sb[:, np_i * CW:(np_i + 1) * CW],
                        start=False, stop=True)
                    # scatter the 4 head-column blocks into vh tiles;
                    # alternate ACT/DVE
                    for hq in range(CW // P):
                        h = np_i * (CW // P) + hq
                        dst = vh[h][:, i * VW: i * VW + P]
                        src = pv[:, hq * P:(hq + 1) * P]
                        if hq % 2 == 0:
                            nc.scalar.activation(
                                dst, src,
                                mybir.ActivationFunctionType.Copy)
                        else:
                            with nc.allow_low_precision(
                                    reason="bf16 v is within budget"):
                                nc.vector.tensor_copy(dst, src)

        # ---------------- phase C: per-head attention (pipelined) ---------
        psB = work_ctx.enter_context(
            tc.tile_pool(name="psB", bufs=1, space="PSUM"))
        att = work_ctx.enter_context(tc.tile_pool(name="att", bufs=2))
        exps_pool = work_ctx.enter_context(tc.tile_pool(name="exps", bufs=28))


        qTs, kTs, exps = {}, {}, {}

        def emit_qk(h):
            wq = att.tile([P, C], mm_dt, tag="wq", bufs=2, name=f"wq{h}")
            nc.sync.dma_start(
                wq[:].rearrange("p (j f) -> p j f", f=P),
                r(wa_3d[:, :, h * P:(h + 1) * P]))
            wk = att.tile([P, C], mm_dt, tag="wk", bufs=2, name=f"wk{h}")
            nc.sync.dma_start(
                wk[:].rearrange("p (j f) -> p j f", f=P),
                r(wa_3d[:, :, C + h * P: C + (h + 1) * P]))
            bq = att.tile([P, 1], F32, tag="bq", name=f"bq{h}")
            nc.sync.dma_start(bq[:], ba_col[h])
            bk = att.tile([P, 1], F32, tag="bk", name=f"bk{h}")
            nc.sync.dma_start(bk[:], ba_col[NH + h])

            qT = att.tile([P, T], at_dt, tag="qT", bufs=1, name=f"qT{h}")
            kT = att.tile([P, T], at_dt, tag="kT", bufs=1, name=f"kT{h}")
            qTs[h], kTs[h] = qT, kT
            for di, (dst, w, bias) in enumerate(
                    ((qT, wq, bq), (kT, wk, bk))):
                pq = [psB.tile([P, 512], F32, tag="pq", bufs=2,
                               name=f"pq{h}_{di}_{ch}")
                      for ch in range(T // 512)]
                for c in range(CT):
                    for ch in range(T // 512):
                        nc.tensor.matmul(
                            pq[ch][:], w[:, c * P:(c + 1) * P],
                            xT[c][:, ch * 512:(ch + 1) * 512],
                            start=(c == 0), stop=(c == CT - 1))
                for ch in range(T // 512):
                    with nc.allow_low_precision(
                            reason="bf16 q/k is within budget"):
                        nc.vector.tensor_scalar_add(
                            dst[:, ch * 512:(ch + 1) * 512], pq[ch][:],
                            bias[:])

        def score_steps(h):
            # one closure per (j, p_i) score tile. Diagonal blocks get
            # halved exp/mask work:
            #   j == 2p_i:   only the left 128 q-cols need masking
            #   j == 2p_i+1: left 128 q-cols are fully masked -> store a
            #                [P, P] tile of the right half only
            qT, kT = qTs[h], kTs[h]
            steps = []
            for j in range(2 * NQP):
                for p_i in range(j // 2, NQP):
                    def step(j=j, p_i=p_i):
                        qs = slice(p_i * QP, (p_i + 1) * QP)
                        sT = psB.tile([P, QP], F32, tag="sT", bufs=2,
                                      name=f"sT{h}_{j}_{p_i}")
                        nc.tensor.matmul(
                            sT[:], kT[:, j * P:(j + 1) * P], qT[:, qs],
                            start=True, stop=True)
                        if j == 2 * p_i + 1:
                            ex = exps_pool.tile([P, P], a8_dt, tag="exps_h",
                                                name=f"ex{h}_{j}_{p_i}")
                            nc.scalar.activation(
                                ex[:], sT[:, P:QP],
                                mybir.ActivationFunctionType.Exp,
                                scale=SM_SCALE)
                            with nc.allow_low_precision(
                                    reason="exact 0/1 mask"):
                                nc.vector.tensor_mul(ex[:], ex[:], maskT[:])
                        else:
                            ex = exps_pool.tile([P, QP], a8_dt, tag="exps",
                                                name=f"ex{h}_{j}_{p_i}")
                            nc.scalar.activation(
                                ex[:], sT[:],
                                mybir.ActivationFunctionType.Exp,
                                scale=SM_SCALE)
                            if j == 2 * p_i:
                                with nc.allow_low_precision(
                                        reason="exact 0/1 mask"):
                                    nc.vector.tensor_mul(
                                        ex[:, 0:P], ex[:, 0:P], maskT[:])
                        exps[(h, j, p_i)] = ex
                    steps.append(step)
            return steps

        y_norms = {}

        def av_steps(h):
            # one closure per q-block: accumulate yacc2[q, 0:128] = raw y,
            # [:, 128] = softmax denom, then normalize on DVE.
            steps = []
            for qb in range(TT):
                def step(qb=qb):
                    p_i, half = qb // 2, (qb % 2) * P
                    yacc2 = psB.tile([P, P + 1], F32, tag="yacc2", bufs=2,
                                     name=f"yacc2_{h}_{qb}")
                    for j in range(qb + 1):
                        ex = exps[(h, j, p_i)]
                        exs = ex[:, 0:P] if ex.shape[1] == P \
                            else ex[:, half:half + P]
                        nc.tensor.matmul(
                            yacc2[:], exs,
                            vh[h][:, j * VW: j * VW + P + 1],
                            start=(j == 0), stop=(j == qb))
                    rden = att.tile([P, 1], F32, tag="rden", bufs=4,
                                    name=f"rden{h}_{qb}")
                    with nc.allow_low_precision(
                            reason="softmax denom recip is within budget"):
                        nc.vector.reciprocal(rden[:], yacc2[:, P:P + 1])
                    y_norm = att.tile([P, P], at_dt, tag="y_norm", bufs=8,
                                      name=f"yn{h}_{qb}")
                    with nc.allow_low_precision(
                            reason="bf16 y is within budget"):
                        nc.vector.tensor_scalar_mul(
                            y_norm[:], yacc2[:, 0:P], rden[:])
                    y_norms[(h, qb)] = y_norm
                steps.append(step)
            return steps

        def emit_ytp(h):
            # PE transposes of the normalized y blocks -> resident yTh
            for qb in range(TT):
                ytp = psB.tile([P, P], at_dt, tag="ytp", bufs=2,
                               name=f"ytp{h}_{qb}")
                nc.tensor.transpose(ytp[:], y_norms[(h, qb)][:], ident_b[:])
                nc.gpsimd.tensor_copy(yTh[h][:, qb * P:(qb + 1) * P], ytp[:])
                del y_norms[(h, qb)]
            for key in [k for k in exps if k[0] == h]:
                del exps[key]

        def interleave(sc, av):
            # spread the (few, chain-heavy) av groups between the (many)
            # score tiles so the PE always has independent matmuls while
            # the ACT engine drains the exp queue
            out, ai = [], 0
            for si, s_step in enumerate(sc):
                out.append(s_step)
                want = (si + 1) * len(av) // len(sc)
                while ai < want:
                    out.append(av[ai])
                    ai += 1
            out.extend(av[ai:])
            return out

        for h in range(NH):
            if h == NH - 1:
                # prefetch the first W_proj tiles while attention drains
                for hh in range(N_PRE):
                    nc.sync.dma_start(wp_pre[hh][:], r(wp_rows[hh]))
            emit_qk(h)
            sc = score_steps(h)
            av = av_steps(h - 1) if h > 0 else []
            for step in interleave(sc, av):
                step()
            if h > 0:
                emit_ytp(h - 1)
        for step in av_steps(NH - 1):
            step()
        emit_ytp(NH - 1)
        work_ctx.close()

        # ---------------- phase D: out = y @ W_proj + b -------------------
        # hh-outer inside 512-wide n-chunks; 8 PSUM banks hold [T, 512].
        with tc.tile_pool(name="psC", bufs=1, space="PSUM") as psC, \
             tc.tile_pool(name="ph3", bufs=1) as ph3:
            wp = list(wp_pre)
            for hh in range(len(wp_pre), NH):
                wpc = ph3.tile([P, C], mm_dt, tag=f"wp{hh}", bufs=1,
                               name=f"wp{hh}")
                nc.sync.dma_start(wpc[:], r(wp_rows[hh]))
                wp.append(wpc)
            for nn in range(4):
                ns = slice(nn * 512, (nn + 1) * 512)
                po = [psC.tile([P, 512], F32, tag=f"po{i}", bufs=1,
                               name=f"po{nn}_{i}")
                      for i in range(TT)]
                for hh in range(NH):
                    for i in range(TT):
                        nc.tensor.matmul(
                            po[i][:], yTh[hh][:, i * P:(i + 1) * P],
                            wp[hh][:, ns],
                            start=(hh == 0), stop=False)
                for i in range(TT):
                    nc.tensor.matmul(
                        po[i][:], ones_row[:], bp_sb[:, ns],
                        start=False, stop=True)
                    osb = ph3.tile([P, 512], F32, tag="osb", bufs=2,
                                   name=f"osb{nn}_{i}")
                    if i % 4 == 1:
                        nc.vector.tensor_copy(osb[:], po[i][:])
                    elif i % 4 == 3:
                        nc.gpsimd.tensor_copy(osb[:], po[i][:])
                    else:
                        nc.scalar.activation(
                            osb[:], po[i][:],
                            mybir.ActivationFunctionType.Copy)
                    nc.sync.dma_start(
                        out_rows[i][:, ns], osb[:])

    return nc


_BUILT = {}


def _get_nc(mode: str):
    if mode not in _BUILT:
        _BUILT[mode] = build_kernel(mode)
    return _BUILT[mode]


def kernel(x, W_attn, b_attn, W_proj, b_proj, mode: str = "f32r", **run_kwargs):
    from concourse.bass_utils import run_bass_kernel_spmd

    x = np.asarray(x, dtype=np.float32)
    W_attn = np.ascontiguousarray(np.asarray(W_attn, dtype=np.float32))
    b_attn = np.ascontiguousarray(np.asarray(b_attn, dtype=np.float32))
    W_proj = np.ascontiguousarray(np.asarray(W_proj, dtype=np.float32))
    b_proj = np.ascontiguousarray(np.asarray(b_proj, dtype=np.float32))

    nc = _get_nc(mode)
    in_maps = [
        {
            "x": np.ascontiguousarray(x[b]),
            "W_attn": W_attn,
            "b_attn": b_attn,
            "W_proj": W_proj,
            "b_proj": b_proj,
        }
        for b in range(N_CORES)
    ]
    res = run_bass_kernel_spmd(nc, in_maps, list(range(N_CORES)), **run_kwargs)
    out = np.stack([res.results[b]["out"] for b in range(N_CORES)], axis=0)
    kernel.last_results = res
    return out


# revision 19
# speedup vs baseline: 1.0053x; 1.0053x over previous
"""Causal self-attention Trainium2 kernel (V2).

Problem: B=8, T=1024, C=2048, 16 heads x 128 head-dim, fp32.
Sharding: data-parallel over batch -- each of the 8 NeuronCores computes one
batch element end-to-end; no collectives.

V2 dataflow (everything resident in SBUF, no DRAM spills):
  phase A+B (pipelined per row-tile): x -> xT (PE transpose, f32r),
    v = x @ W_v + b_v into per-head SBUF tiles vh[h] (bf16), laid out
    [k-part, d] per 128-k-block with a ones column appended per block.
  phase C per head (software-pipelined, iteration h emits):
    qk(h):   qT/kT[d,T] bf16 <- ACT(bias) <- PE(W_qk^T @ xT)
    sc(h):   sT[k,q]    <- kT-block^T-free @ qT 256-chunk; exps = Exp (ACT,
             bf16), causal-masked (DVE) on diagonal blocks
    av(h-1): yacc2[q,130] (PSUM) += exps-128-slice^T-free @ [v|1] (moving 129)
             -> col 128 = softmax denominator; rden = 1/den (DVE);
             y_norm[q,d] = yacc2 * rden (DVE tensor_scalar, per-partition);
             yT via PE transpose (bf16) -> yTh[h] resident SBUF
  phase D: out = y @ W_proj + b (hh-outer over 512-wide n-chunks, 8 PSUM
    banks hold [T, 512]; lhsT = yTh slices (bf16), rhs = wp (f32r), so the
    first chunk can start as soon as wp[0] is DMA'd; no spill reload).
"""

import math
from contextlib import ExitStack

import numpy as np

import concourse.bass as bass
import concourse.mybir as mybir
import concourse.tile as tile
from concourse.masks import make_identity
from concourse.vector_clock import ScopedClock

F32 = mybir.dt.float32
F32R = mybir.dt.float32r
BF16 = mybir.dt.bfloat16
F8E4 = mybir.dt.float8e4

B, T, C = 8, 1024, 2048
NH, HD = 16, 128
P = 128
TT = T // P            # 8 row tiles
CT = C // P            # 16 channel tiles
QP = 256               # q-pair width for score matmuls
NQP = T // QP          # 4 q-pairs
VW = 130               # per-k-block stride in vh tiles: 128 v cols + ones col
SM_SCALE = 1.0 / math.sqrt(HD)

N_CORES = 8

# --------------------------------------------------------------------------
# Walrus workaround: this container's walrus rejects any instruction with
# more than one sync wait command. Split multi-wait instructions into a
# chain of single-wait NoOps/Drains on the same engine (engine queues
# process waits in order, so semantics are unchanged).
# --------------------------------------------------------------------------
_orig_commit_instruction = tile.TileContext._commit_instruction


def _patched_commit_instruction(self, inst, lazy_reg_writes=True):
    si = inst.sync_info
    if (
        si is not None
        and len(si.on_wait) > 1
        and inst.engine != mybir.EngineType.Unassigned
    ):
        waits = list(si.on_wait)
        for w in waits[:-1]:
            nop = mybir.InstNoOp(
                name=self.nc.get_next_instruction_name(),
                engine=inst.engine,
                bass_nofuse=True,
                sync_info=mybir.SyncInfo(on_wait=[w], on_update=[]),
            )
            _orig_commit_instruction(self, nop, lazy_reg_writes=False)
        inst.sync_info = mybir.SyncInfo(
            on_wait=[waits[-1]], on_update=list(si.on_update)
        )
    return _orig_commit_instruction(self, inst, lazy_reg_writes=lazy_reg_writes)


def _patched_drain_and_barrier(self, tick_clock, wait_clock):
    drain_inst = self.nc.sync.drain()
    wait_clock.add_sem_waits(
        drain_inst.ins, ScopedClock({None: tick_clock.global_clock})
    )
    si = drain_inst.ins.sync_info
    if si is not None and len(si.on_wait) > 1:
        waits = list(si.on_wait)
        drain_inst.ins.sync_info = mybir.SyncInfo(
            on_wait=[waits[0]], on_update=list(si.on_update)
        )
        for w in waits[1:]:
            d2 = self.nc.sync.drain()
            d2.ins.sync_info = mybir.SyncInfo(on_wait=[w], on_update=[])
    self.nc.all_engine_barrier()
    assert self.sems is not None
    popped = self.nc._tile_sem_poison_stack.pop()
    assert popped is self._sem_poison
    self.nc.clear_and_free_semaphores(list(self.sems.allocated().values()))
    self.nc.all_engine_barrier()


def _apply_patches():
    tile.TileContext._commit_instruction = _patched_commit_instruction
    tile.TileContext._drain_and_barrier = _patched_drain_and_barrier


# --------------------------------------------------------------------------
# Kernel builder
# --------------------------------------------------------------------------

def build_kernel(mode: str = "f32r") -> bass.Bass:
    """mode: 'f32r' (fast path: f32r projections, bf16 attention) or
    'f32' (full fp32 matmuls, slower; debugging)."""
    _apply_patches()
    mm_dt = F32R if mode == "f32r" else F32
    at_dt = BF16 if mode == "f32r" else F32
    a8_dt = F8E4 if mode == "f32r" else F32

    nc = bass.Bass("TRN2", target_bir_lowering=False, debug=False)

    x_ap = nc.dram_tensor("x", [T, C], F32, kind="ExternalInput").ap()
    wa_ap = nc.dram_tensor("W_attn", [C, 3 * C], F32, kind="ExternalInput").ap()
    ba_ap = nc.dram_tensor("b_attn", [3 * C], F32, kind="ExternalInput").ap()
    wp_ap = nc.dram_tensor("W_proj", [C, C], F32, kind="ExternalInput").ap()
    bp_ap = nc.dram_tensor("b_proj", [C], F32, kind="ExternalInput").ap()
    out_ap = nc.dram_tensor("out", [T, C], F32, kind="ExternalOutput").ap()

    def r(ap):
        return ap.bitcast(mm_dt) if mm_dt is F32R else ap

    # DRAM views
    x_rows = x_ap.rearrange("(i p) c -> i p c", p=P)          # [TT, P, C]
    out_rows = out_ap.rearrange("(i p) c -> i p c", p=P)      # [TT, P, C]
    wa_3d = wa_ap.rearrange("(j p) n -> p j n", p=P)          # [P, CT, 3C]
    wp_rows = wp_ap.rearrange("(h p) n -> h p n", p=P)        # [NH, P, C]
    ba_col = ba_ap.rearrange("(n p one) -> n p one", p=P, one=1)  # [48, P, 1]
    bv_row = ba_ap.rearrange("(n c) -> n c", n=3)             # [3, C]
    bp_row = bp_ap.rearrange("(one c) -> one c", one=1)       # [1, C]

    with tile.TileContext(nc) as tc, ExitStack() as ctx:
        # ---------------- constants ----------------
        const = ctx.enter_context(tc.tile_pool(name="const", bufs=1))
        ident = const.tile([P, P], mm_dt)
        make_identity(nc, ident[:])
        ident_b = const.tile([P, P], at_dt)
        make_identity(nc, ident_b[:])
        # lower-triangular causal mask for diagonal k-blocks:
        # maskT[k, q] = 1 if q >= k else 0 (both diagonal cases reduce to it)
        maskT = const.tile([P, P], a8_dt)
        nc.gpsimd.memset(maskT[:], 1.0)
        nc.gpsimd.affine_select(
            out=maskT[:], in_=maskT[:], compare_op=mybir.AluOpType.is_ge,
            fill=0.0, base=0, pattern=[[1, P]], channel_multiplier=-1)
        # ones row (K=1 bias matmul lhsT)
        ones_row_f = const.tile([1, P], F32)
        nc.vector.memset(ones_row_f[:], 1.0)
        ones_row = const.tile([1, P], mm_dt)
        nc.vector.tensor_copy(ones_row[:], ones_row_f[:])

        # ---------------- resident tensors ----------------
        # yTh spans phases C-D; xT/vh close after attention (work_ctx).
        res_pool = ctx.enter_context(tc.tile_pool(name="resident", bufs=1))
        yTh = [res_pool.tile([P, T], at_dt, tag=f"yTh{h}", name=f"yTh{h}")
               for h in range(NH)]
        # prefetch targets for phase D (W_proj head tiles + bias); the pool
        # must outlive work_ctx, so it is opened here. DMAs for wp_pre are
        # emitted late (during the last attention head).
        pre_pool = ctx.enter_context(tc.tile_pool(name="pre", bufs=1))
        bp_sb = pre_pool.tile([1, C], mm_dt, tag="bp")
        nc.sync.dma_start(bp_sb[:], r(bp_row[:, :]))
        N_PRE = 5
        wp_pre = [pre_pool.tile([P, C], mm_dt, tag=f"wpp{hh}", name=f"wpp{hh}")
                  for hh in range(N_PRE)]
        work_ctx = ExitStack()
        work = work_ctx.enter_context(tc.tile_pool(name="work", bufs=1))
        xT = [work.tile([P, T], at_dt, tag=f"xT{j}", name=f"xT{j}")
              for j in range(CT)]
        # vh[h]: per k-block j, cols [j*VW, j*VW+128) = v rows of block j for
        # head h; col j*VW+128 = 1.0 (softmax denominator); col +129 unused.
        vh = [work.tile([P, TT * VW], a8_dt, tag=f"vh{h}", name=f"vh{h}")
              for h in range(NH)]
        for h in range(NH):
            # split memsets across Pool and DVE so neither gates the start
            if h % 2 == 0:
                nc.gpsimd.memset(vh[h][:], 1.0)
            else:
                nc.vector.memset(vh[h][:], 1.0)

        # ---------------- phase A: x -> xT --------------------------------
        NW = 4                           # n-chunks of W_v (512 wide each)
        CW = C // NW
        with tc.tile_pool(name="psA", bufs=1, space="PSUM") as psA, \
             tc.tile_pool(name="phA", bufs=1) as phA:
            for i in range(TT):
                xa = phA.tile([P, C], mm_dt, tag="xa", bufs=4,
                              name=f"xa{i}")
                nc.sync.dma_start(xa[:], r(x_rows[i]))
                for j in range(CT):
                    tp = psA.tile([P, P], mm_dt, tag="tp", bufs=4)
                    nc.tensor.transpose(tp[:], xa[:, j * P:(j + 1) * P],
                                        ident[:])
                    # rotate evacuation across ACT/DVE/Pool (f32 -> bf16)
                    dst = xT[j][:, i * P:(i + 1) * P]
                    if j % 4 == 0:
                        with nc.allow_low_precision(
                                reason="bf16 x is within budget"):
                            nc.vector.tensor_copy(dst, tp[:].bitcast(F32))
                    elif j % 4 == 2:
                        with nc.allow_low_precision(
                                reason="bf16 x is within budget"):
                            nc.gpsimd.tensor_copy(dst, tp[:].bitcast(F32))
                    else:
                        nc.scalar.activation(
                            dst, tp[:].bitcast(F32),
                            mybir.ActivationFunctionType.Copy)

        # ---------------- phase B: v = x @ W_v + b_v ----------------------
        with tc.tile_pool(name="psA2", bufs=1, space="PSUM") as psA, \
             tc.tile_pool(name="phB", bufs=1) as phB:
            bv_sb = phB.tile([1, C], mm_dt, tag="bv")
            nc.sync.dma_start(bv_sb[:], r(bv_row[2:3, :]))
            wv = {}
            for np_i in range(NW):
                for c in range(CT):
                    wvc = phB.tile([P, CW], mm_dt, tag=f"wv{c}", bufs=1,
                                   name=f"wv{c}_{np_i}")
                    nc.sync.dma_start(
                        wvc[:],
                        r(wa_3d[:, c,
                                2 * C + np_i * CW: 2 * C + (np_i + 1) * CW]))
                    wv[(np_i, c)] = wvc
            for np_i in range(NW):
                for i in range(TT):
                    pv = psA.tile([P, CW], F32, tag="pv", bufs=4,
                                  name=f"pv{np_i}_{i}")
                    for c in range(CT):
                        nc.tensor.matmul(
                            pv[:], xT[c][:, i * P:(i + 1) * P],
                            wv[(np_i, c)][:],
                            start=(c == 0), stop=False)
                    nc.tensor.matmul(
                        pv[:], ones_row[:],
                        bv_sb[:, np_i * CW:(np_i + 1) * CW],
                        start=False, stop=True)
                    # scatter the 4 head-column blocks into vh tiles;
                    # alternate ACT/DVE
                    for hq in range(CW // P):
                        h = np_i * (CW // P) + hq
                        dst = vh[h][:, i * VW: i * VW + P]
                        src = pv[:, hq * P:(hq + 1) * P]
                        if hq % 2 == 0:
                            nc.scalar.activation(
                                dst, src,
                                mybir.ActivationFunctionType.Copy)
                        else:
                            with nc.allow_low_precision(
                                    reason="bf16 v is within budget"):
                                nc.vector.tensor_copy(dst, src)

        # ---------------- phase C: per-head attention (pipelined) ---------
        psB = work_ctx.enter_context(
            tc.tile_pool(name="psB", bufs=1, space="PSUM"))
        att = work_ctx.enter_context(tc.tile_pool(name="att", bufs=2))
        exps_pool = work_ctx.enter_context(tc.tile_pool(name="exps", bufs=28))


        qTs, kTs, exps = {}, {}, {}

        def emit_qk(h):
            wq = att.tile([P, C], mm_dt, tag="wq", bufs=2, name=f"wq{h}")
            nc.sync.dma_start(
                wq[:].rearrange("p (j f) -> p j f", f=P),
                r(wa_3d[:, :, h * P:(h + 1) * P]))
            wk = att.tile([P, C], mm_dt, tag="wk", bufs=2, name=f"wk{h}")
            nc.sync.dma_start(
                wk[:].rearrange("p (j f) -> p j f", f=P),
                r(wa_3d[:, :, C + h * P: C + (h + 1) * P]))
            bq = att.tile([P, 1], F32, tag="bq", name=f"bq{h}")
            nc.sync.dma_start(bq[:], ba_col[h])
            bk = att.tile([P, 1], F32, tag="bk", name=f"bk{h}")
            nc.sync.dma_start(bk[:], ba_col[NH + h])

            qT = att.tile([P, T], at_dt, tag="qT", bufs=1, name=f"qT{h}")
            kT = att.tile([P, T], at_dt, tag="kT", bufs=1, name=f"kT{h}")
            qTs[h], kTs[h] = qT, kT
            for di, (dst, w, bias) in enumerate(
                    ((qT, wq, bq), (kT, wk, bk))):
                pq = [psB.tile([P, 512], F32, tag="pq", bufs=2,
                               name=f"pq{h}_{di}_{ch}")
                      for ch in range(T // 512)]
                for c in range(CT):
                    for ch in range(T // 512):
                        nc.tensor.matmul(
                            pq[ch][:], w[:, c * P:(c + 1) * P],
                            xT[c][:, ch * 512:(ch + 1) * 512],
                            start=(c == 0), stop=(c == CT - 1))
                for ch in range(T // 512):
                    nc.scalar.activation(
                        dst[:, ch * 512:(ch + 1) * 512], pq[ch][:],
                        mybir.ActivationFunctionType.Identity,
                        bias=bias[:])

        def score_steps(h):
            # one closure per (j, p_i) score tile. Diagonal blocks get
            # halved exp/mask work:
            #   j == 2p_i:   only the left 128 q-cols need masking
            #   j == 2p_i+1: left 128 q-cols are fully masked -> store a
            #                [P, P] tile of the right half only
            qT, kT = qTs[h], kTs[h]
            steps = []
            for j in range(2 * NQP):
                for p_i in range(j // 2, NQP):
                    def step(j=j, p_i=p_i):
                        qs = slice(p_i * QP, (p_i + 1) * QP)
                        sT = psB.tile([P, QP], F32, tag="sT", bufs=2,
                                      name=f"sT{h}_{j}_{p_i}")
                        nc.tensor.matmul(
                            sT[:], kT[:, j * P:(j + 1) * P], qT[:, qs],
                            start=True, stop=True)
                        if j == 2 * p_i + 1:
                            ex = exps_pool.tile([P, P], a8_dt, tag="exps_h",
                                                name=f"ex{h}_{j}_{p_i}")
                            nc.scalar.activation(
                                ex[:], sT[:, P:QP],
                                mybir.ActivationFunctionType.Exp,
                                scale=SM_SCALE)
                            with nc.allow_low_precision(
                                    reason="exact 0/1 mask"):
                                nc.vector.tensor_mul(ex[:], ex[:], maskT[:])
                        else:
                            ex = exps_pool.tile([P, QP], a8_dt, tag="exps",
                                                name=f"ex{h}_{j}_{p_i}")
                            nc.scalar.activation(
                                ex[:], sT[:],
                                mybir.ActivationFunctionType.Exp,
                                scale=SM_SCALE)
                            if j == 2 * p_i:
                                with nc.allow_low_precision(
                                        reason="exact 0/1 mask"):
                                    nc.vector.tensor_mul(
                                        ex[:, 0:P], ex[:, 0:P], maskT[:])
                        exps[(h, j, p_i)] = ex
                    steps.append(step)
            return steps

        y_norms = {}

        def av_steps(h):
            # one closure per q-block: accumulate yacc2[q, 0:128] = raw y,
            # [:, 128] = softmax denom, then normalize on DVE.
            steps = []
            for qb in range(TT):
                def step(qb=qb):
                    p_i, half = qb // 2, (qb % 2) * P
                    yacc2 = psB.tile([P, P + 1], F32, tag="yacc2", bufs=2,
                                     name=f"yacc2_{h}_{qb}")
                    for j in range(qb + 1):
                        ex = exps[(h, j, p_i)]
                        exs = ex[:, 0:P] if ex.shape[1] == P \
                            else ex[:, half:half + P]
                        nc.tensor.matmul(
                            yacc2[:], exs,
                            vh[h][:, j * VW: j * VW + P + 1],
                            start=(j == 0), stop=(j == qb))
                    rden = att.tile([P, 1], F32, tag="rden", bufs=4,
                                    name=f"rden{h}_{qb}")
                    with nc.allow_low_precision(
                            reason="softmax denom recip is within budget"):
                        nc.vector.reciprocal(rden[:], yacc2[:, P:P + 1])
                    y_norm = att.tile([P, P], at_dt, tag="y_norm", bufs=8,
                                      name=f"yn{h}_{qb}")
                    with nc.allow_low_precision(
                            reason="bf16 y is within budget"):
                        nc.vector.tensor_scalar_mul(
                            y_norm[:], yacc2[:, 0:P], rden[:])
                    y_norms[(h, qb)] = y_norm
                steps.append(step)
            return steps

        def emit_ytp(h):
            # PE transposes of the normalized y blocks -> resident yTh
            for qb in range(TT):
                ytp = psB.tile([P, P], at_dt, tag="ytp", bufs=2,
                               name=f"ytp{h}_{qb}")
                nc.tensor.transpose(ytp[:], y_norms[(h, qb)][:], ident_b[:])
                nc.gpsimd.tensor_copy(yTh[h][:, qb * P:(qb + 1) * P], ytp[:])
                del y_norms[(h, qb)]
            for key in [k for k in exps if k[0] == h]:
                del exps[key]

        def interleave(sc, av):
            # spread the (few, chain-heavy) av groups between the (many)
            # score tiles so the PE always has independent matmuls while
            # the ACT engine drains the exp queue
            out, ai = [], 0
            for si, s_step in enumerate(sc):
                out.append(s_step)
                want = (si + 1) * len(av) // len(sc)
                while ai < want:
                    out.append(av[ai])
                    ai += 1
            out.extend(av[ai:])
            return out

        for h in range(NH):
            if h == NH - 1:
                # prefetch the first W_proj tiles while attention drains
                for hh in range(N_PRE):
                    nc.sync.dma_start(wp_pre[hh][:], r(wp_rows[hh]))
            emit_qk(h)
            sc = score_steps(h)
            av = av_steps(h - 1) if h > 0 else []
            for step in interleave(sc, av):
                step()
            if h > 0:
                emit_ytp(h - 1)
        for step in av_steps(NH - 1):
            step()
        emit_ytp(NH - 1)
        work_ctx.close()

        # ---------------- phase D: out = y @ W_proj + b -------------------
        # hh-outer inside 512-wide n-chunks; 8 PSUM banks hold [T, 512].
        with tc.tile_pool(name="psC", bufs=1, space="PSUM") as psC, \
             tc.tile_pool(name="ph3", bufs=1) as ph3:
            wp = list(wp_pre)
            for hh in range(len(wp_pre), NH):
                wpc = ph3.tile([P, C], mm_dt, tag=f"wp{hh}", bufs=1,
                               name=f"wp{hh}")
                nc.sync.dma_start(wpc[:], r(wp_rows[hh]))
                wp.append(wpc)
            for nn in range(4):
                ns = slice(nn * 512, (nn + 1) * 512)
                po = [psC.tile([P, 512], F32, tag=f"po{i}", bufs=1,
                               name=f"po{nn}_{i}")
                      for i in range(TT)]
                for hh in range(NH):
                    for i in range(TT):
                        nc.tensor.matmul(
                            po[i][:], yTh[hh][:, i * P:(i + 1) * P],
                            wp[hh][:, ns],
                            start=(hh == 0), stop=False)
                for i in range(TT):
                    nc.tensor.matmul(
                        po[i][:], ones_row[:], bp_sb[:, ns],
                        start=False, stop=True)
                    osb = ph3.tile([P, 512], F32, tag="osb", bufs=2,
                                   name=f"osb{nn}_{i}")
                    if i % 4 == 1:
                        nc.vector.tensor_copy(osb[:], po[i][:])
                    elif i % 4 == 3:
                        nc.gpsimd.tensor_copy(osb[:], po[i][:])
                    else:
                        nc.scalar.activation(
                            osb[:], po[i][:],
                            mybir.ActivationFunctionType.Copy)
                    nc.sync.dma_start(
                        out_rows[i][:, ns], osb[:])

    return nc


_BUILT = {}


def _get_nc(mode: str):
    if mode not in _BUILT:
        _BUILT[mode] = build_kernel(mode)
    return _BUILT[mode]


def kernel(x, W_attn, b_attn, W_proj, b_proj, mode: str = "f32r", **run_kwargs):
    from concourse.bass_utils import run_bass_kernel_spmd

    x = np.asarray(x, dtype=np.float32)
    W_attn = np.ascontiguousarray(np.asarray(W_attn, dtype=np.float32))
    b_attn = np.ascontiguousarray(np.asarray(b_attn, dtype=np.float32))
    W_proj = np.ascontiguousarray(np.asarray(W_proj, dtype=np.float32))
    b_proj = np.ascontiguousarray(np.asarray(b_proj, dtype=np.float32))

    nc = _get_nc(mode)
    in_maps = [
        {
            "x": np.ascontiguousarray(x[b]),
            "W_attn": W_attn,
            "b_attn": b_attn,
            "W_proj": W_proj,
            "b_proj": b_proj,
        }
        for b in range(N_CORES)
    ]
    res = run_bass_kernel_spmd(nc, in_maps, list(range(N_CORES)), **run_kwargs)
    out = np.stack([res.results[b]["out"] for b in range(N_CORES)], axis=0)
    kernel.last_results = res
    return out


# revision 20
# speedup vs baseline: 1.0139x; 1.0086x over previous
"""Causal self-attention Trainium2 kernel (V2).

Problem: B=8, T=1024, C=2048, 16 heads x 128 head-dim, fp32.
Sharding: data-parallel over batch -- each of the 8 NeuronCores computes one
batch element end-to-end; no collectives.

V2 dataflow (everything resident in SBUF, no DRAM spills):
  phase A+B (pipelined per row-tile): x -> xT (PE transpose, f32r),
    v = x @ W_v + b_v into per-head SBUF tiles vh[h] (bf16), laid out
    [k-part, d] per 128-k-block with a ones column appended per block.
  phase C per head (software-pipelined, iteration h emits):
    qk(h):   qT/kT[d,T] bf16 <- ACT(bias) <- PE(W_qk^T @ xT)
    sc(h):   sT[k,q]    <- kT-block^T-free @ qT 256-chunk; exps = Exp (ACT,
             bf16), causal-masked (DVE) on diagonal blocks
    av(h-1): yacc2[q,130] (PSUM) += exps-128-slice^T-free @ [v|1] (moving 129)
             -> col 128 = softmax denominator; rden = 1/den (DVE);
             y_norm[q,d] = yacc2 * rden (DVE tensor_scalar, per-partition);
             yT via PE transpose (bf16) -> yTh[h] resident SBUF
  phase D: out = y @ W_proj + b (hh-outer over 512-wide n-chunks, 8 PSUM
    banks hold [T, 512]; lhsT = yTh slices (bf16), rhs = wp (f32r), so the
    first chunk can start as soon as wp[0] is DMA'd; no spill reload).
"""

import math
from contextlib import ExitStack

import numpy as np

import concourse.bass as bass
import concourse.mybir as mybir
import concourse.tile as tile
from concourse.masks import make_identity
from concourse.vector_clock import ScopedClock

F32 = mybir.dt.float32
F32R = mybir.dt.float32r
BF16 = mybir.dt.bfloat16
F8E4 = mybir.dt.float8e4

B, T, C = 8, 1024, 2048
NH, HD = 16, 128
P = 128
TT = T // P            # 8 row tiles
CT = C // P            # 16 channel tiles
QP = 256               # q-pair width for score matmuls
NQP = T // QP          # 4 q-pairs
VW = 130               # per-k-block stride in vh tiles: 128 v cols + ones col
SM_SCALE = 1.0 / math.sqrt(HD)

N_CORES = 8

# --------------------------------------------------------------------------
# Walrus workaround: this container's walrus rejects any instruction with
# more than one sync wait command. Split multi-wait instructions into a
# chain of single-wait NoOps/Drains on the same engine (engine queues
# process waits in order, so semantics are unchanged).
# --------------------------------------------------------------------------
_orig_commit_instruction = tile.TileContext._commit_instruction


def _patched_commit_instruction(self, inst, lazy_reg_writes=True):
    si = inst.sync_info
    if (
        si is not None
        and len(si.on_wait) > 1
        and inst.engine != mybir.EngineType.Unassigned
    ):
        waits = list(si.on_wait)
        for w in waits[:-1]:
            nop = mybir.InstNoOp(
                name=self.nc.get_next_instruction_name(),
                engine=inst.engine,
                bass_nofuse=True,
                sync_info=mybir.SyncInfo(on_wait=[w], on_update=[]),
            )
            _orig_commit_instruction(self, nop, lazy_reg_writes=False)
        inst.sync_info = mybir.SyncInfo(
            on_wait=[waits[-1]], on_update=list(si.on_update)
        )
    return _orig_commit_instruction(self, inst, lazy_reg_writes=lazy_reg_writes)


def _patched_drain_and_barrier(self, tick_clock, wait_clock):
    drain_inst = self.nc.sync.drain()
    wait_clock.add_sem_waits(
        drain_inst.ins, ScopedClock({None: tick_clock.global_clock})
    )
    si = drain_inst.ins.sync_info
    if si is not None and len(si.on_wait) > 1:
        waits = list(si.on_wait)
        drain_inst.ins.sync_info = mybir.SyncInfo(
            on_wait=[waits[0]], on_update=list(si.on_update)
        )
        for w in waits[1:]:
            d2 = self.nc.sync.drain()
            d2.ins.sync_info = mybir.SyncInfo(on_wait=[w], on_update=[])
    self.nc.all_engine_barrier()
    assert self.sems is not None
    popped = self.nc._tile_sem_poison_stack.pop()
    assert popped is self._sem_poison
    self.nc.clear_and_free_semaphores(list(self.sems.allocated().values()))
    self.nc.all_engine_barrier()


def _apply_patches():
    tile.TileContext._commit_instruction = _patched_commit_instruction
    tile.TileContext._drain_and_barrier = _patched_drain_and_barrier


# --------------------------------------------------------------------------
# Kernel builder
# --------------------------------------------------------------------------

def build_kernel(mode: str = "f32r") -> bass.Bass:
    """mode: 'f32r' (fast path: f32r projections, bf16 attention) or
    'f32' (full fp32 matmuls, slower; debugging)."""
    _apply_patches()
    mm_dt = F32R if mode == "f32r" else F32
    at_dt = BF16 if mode == "f32r" else F32
    a8_dt = F8E4 if mode == "f32r" else F32

    nc = bass.Bass("TRN2", target_bir_lowering=False, debug=False)

    x_ap = nc.dram_tensor("x", [T, C], F32, kind="ExternalInput").ap()
    wa_ap = nc.dram_tensor("W_attn", [C, 3 * C], F32, kind="ExternalInput").ap()
    ba_ap = nc.dram_tensor("b_attn", [3 * C], F32, kind="ExternalInput").ap()
    wp_ap = nc.dram_tensor("W_proj", [C, C], F32, kind="ExternalInput").ap()
    bp_ap = nc.dram_tensor("b_proj", [C], F32, kind="ExternalInput").ap()
    out_ap = nc.dram_tensor("out", [T, C], F32, kind="ExternalOutput").ap()

    def r(ap):
        return ap.bitcast(mm_dt) if mm_dt is F32R else ap

    # DRAM views
    x_rows = x_ap.rearrange("(i p) c -> i p c", p=P)          # [TT, P, C]
    out_rows = out_ap.rearrange("(i p) c -> i p c", p=P)      # [TT, P, C]
    wa_3d = wa_ap.rearrange("(j p) n -> p j n", p=P)          # [P, CT, 3C]
    wp_rows = wp_ap.rearrange("(h p) n -> h p n", p=P)        # [NH, P, C]
    ba_col = ba_ap.rearrange("(n p one) -> n p one", p=P, one=1)  # [48, P, 1]
    bv_row = ba_ap.rearrange("(n c) -> n c", n=3)             # [3, C]
    bp_row = bp_ap.rearrange("(one c) -> one c", one=1)       # [1, C]

    with tile.TileContext(nc) as tc, ExitStack() as ctx:
        # ---------------- constants ----------------
        const = ctx.enter_context(tc.tile_pool(name="const", bufs=1))
        ident = const.tile([P, P], mm_dt)
        make_identity(nc, ident[:])
        ident_b = const.tile([P, P], at_dt)
        make_identity(nc, ident_b[:])
        # lower-triangular causal mask for diagonal k-blocks:
        # maskT[k, q] = 1 if q >= k else 0 (both diagonal cases reduce to it)
        maskT = const.tile([P, P], a8_dt)
        nc.gpsimd.memset(maskT[:], 1.0)
        nc.gpsimd.affine_select(
            out=maskT[:], in_=maskT[:], compare_op=mybir.AluOpType.is_ge,
            fill=0.0, base=0, pattern=[[1, P]], channel_multiplier=-1)
        # ones row (K=1 bias matmul lhsT)
        ones_row_f = const.tile([1, P], F32)
        nc.vector.memset(ones_row_f[:], 1.0)
        ones_row = const.tile([1, P], mm_dt)
        nc.vector.tensor_copy(ones_row[:], ones_row_f[:])

        # ---------------- resident tensors ----------------
        # yTh spans phases C-D; xT/vh close after attention (work_ctx).
        res_pool = ctx.enter_context(tc.tile_pool(name="resident", bufs=1))
        yTh = [res_pool.tile([P, T], at_dt, tag=f"yTh{h}", name=f"yTh{h}")
               for h in range(NH)]
        # prefetch targets for phase D (W_proj head tiles + bias); the pool
        # must outlive work_ctx, so it is opened here. DMAs for wp_pre are
        # emitted late (during the last attention head).
        pre_pool = ctx.enter_context(tc.tile_pool(name="pre", bufs=1))
        bp_sb = pre_pool.tile([1, C], mm_dt, tag="bp")
        nc.sync.dma_start(bp_sb[:], r(bp_row[:, :]))
        N_PRE = 5
        wp_pre = [pre_pool.tile([P, C], mm_dt, tag=f"wpp{hh}", name=f"wpp{hh}")
                  for hh in range(N_PRE)]
        work_ctx = ExitStack()
        work = work_ctx.enter_context(tc.tile_pool(name="work", bufs=1))
        xT = [work.tile([P, T], at_dt, tag=f"xT{j}", name=f"xT{j}")
              for j in range(CT)]
        # vh[h]: per k-block j, cols [j*VW, j*VW+128) = v rows of block j for
        # head h; col j*VW+128 = 1.0 (softmax denominator); col +129 unused.
        vh = [work.tile([P, TT * VW], a8_dt, tag=f"vh{h}", name=f"vh{h}")
              for h in range(NH)]
        for h in range(NH):
            # split memsets across Pool and DVE so neither gates the start
            if h % 2 == 0:
                nc.gpsimd.memset(vh[h][:], 1.0)
            else:
                nc.vector.memset(vh[h][:], 1.0)

        # ---------------- phase A: x -> xT --------------------------------
        NW = 4                           # n-chunks of W_v (512 wide each)
        CW = C // NW
        with tc.tile_pool(name="psA", bufs=1, space="PSUM") as psA, \
             tc.tile_pool(name="phA", bufs=1) as phA:
            for i in range(TT):
                xa = phA.tile([P, C], mm_dt, tag="xa", bufs=4,
                              name=f"xa{i}")
                nc.sync.dma_start(xa[:], r(x_rows[i]))
                for j in range(CT):
                    tp = psA.tile([P, P], mm_dt, tag="tp", bufs=4)
                    nc.tensor.transpose(tp[:], xa[:, j * P:(j + 1) * P],
                                        ident[:])
                    # rotate evacuation across ACT/DVE/Pool (f32 -> bf16)
                    dst = xT[j][:, i * P:(i + 1) * P]
                    if j % 4 == 0:
                        with nc.allow_low_precision(
                                reason="bf16 x is within budget"):
                            nc.vector.tensor_copy(dst, tp[:].bitcast(F32))
                    elif j % 4 == 2:
                        with nc.allow_low_precision(
                                reason="bf16 x is within budget"):
                            nc.gpsimd.tensor_copy(dst, tp[:].bitcast(F32))
                    else:
                        nc.scalar.activation(
                            dst, tp[:].bitcast(F32),
                            mybir.ActivationFunctionType.Copy)

        # ---------------- phase B: v = x @ W_v + b_v ----------------------
        with tc.tile_pool(name="psA2", bufs=1, space="PSUM") as psA, \
             tc.tile_pool(name="phB", bufs=1) as phB:
            bv_sb = phB.tile([1, C], mm_dt, tag="bv")
            nc.sync.dma_start(bv_sb[:], r(bv_row[2:3, :]))
            wv = {}
            for np_i in range(NW):
                for c in range(CT):
                    wvc = phB.tile([P, CW], mm_dt, tag=f"wv{c}", bufs=1,
                                   name=f"wv{c}_{np_i}")
                    nc.sync.dma_start(
                        wvc[:],
                        r(wa_3d[:, c,
                                2 * C + np_i * CW: 2 * C + (np_i + 1) * CW]))
                    wv[(np_i, c)] = wvc
            for np_i in range(NW):
                for i in range(TT):
                    pv = psA.tile([P, CW], F32, tag="pv", bufs=4,
                                  name=f"pv{np_i}_{i}")
                    for c in range(CT):
                        nc.tensor.matmul(
                            pv[:], xT[c][:, i * P:(i + 1) * P],
                            wv[(np_i, c)][:],
                            start=(c == 0), stop=False)
                    nc.tensor.matmul(
                        pv[:], ones_row[:],
                        bv_sb[:, np_i * CW:(np_i + 1) * CW],
                        start=False, stop=True)
                    # scatter the 4 head-column blocks into vh tiles;
                    # alternate ACT/DVE
                    for hq in range(CW // P):
                        h = np_i * (CW // P) + hq
                        dst = vh[h][:, i * VW: i * VW + P]
                        src = pv[:, hq * P:(hq + 1) * P]
                        if hq % 2 == 0:
                            nc.scalar.activation(
                                dst, src,
                                mybir.ActivationFunctionType.Copy)
                        else:
                            with nc.allow_low_precision(
                                    reason="bf16 v is within budget"):
                                nc.vector.tensor_copy(dst, src)

        # ---------------- phase C: per-head attention (pipelined) ---------
        psB = work_ctx.enter_context(
            tc.tile_pool(name="psB", bufs=1, space="PSUM"))
        att = work_ctx.enter_context(tc.tile_pool(name="att", bufs=2))
        exps_pool = work_ctx.enter_context(tc.tile_pool(name="exps", bufs=28))


        qTs, kTs, exps = {}, {}, {}

        def emit_qk(h):
            wq = att.tile([P, C], mm_dt, tag="wq", bufs=2, name=f"wq{h}")
            nc.sync.dma_start(
                wq[:].rearrange("p (j f) -> p j f", f=P),
                r(wa_3d[:, :, h * P:(h + 1) * P]))
            wk = att.tile([P, C], mm_dt, tag="wk", bufs=2, name=f"wk{h}")
            nc.sync.dma_start(
                wk[:].rearrange("p (j f) -> p j f", f=P),
                r(wa_3d[:, :, C + h * P: C + (h + 1) * P]))
            bq = att.tile([P, 1], F32, tag="bq", name=f"bq{h}")
            nc.sync.dma_start(bq[:], ba_col[h])
            bk = att.tile([P, 1], F32, tag="bk", name=f"bk{h}")
            nc.sync.dma_start(bk[:], ba_col[NH + h])

            qT = att.tile([P, T], at_dt, tag="qT", bufs=1, name=f"qT{h}")
            kT = att.tile([P, T], at_dt, tag="kT", bufs=1, name=f"kT{h}")
            qTs[h], kTs[h] = qT, kT
            for di, (dst, w, bias) in enumerate(
                    ((qT, wq, bq), (kT, wk, bk))):
                pq = [psB.tile([P, 512], F32, tag="pq", bufs=2,
                               name=f"pq{h}_{di}_{ch}")
                      for ch in range(T // 512)]
                for c in range(CT):
                    for ch in range(T // 512):
                        nc.tensor.matmul(
                            pq[ch][:], w[:, c * P:(c + 1) * P],
                            xT[c][:, ch * 512:(ch + 1) * 512],
                            start=(c == 0), stop=(c == CT - 1))
                for ch in range(T // 512):
                    nc.scalar.activation(
                        dst[:, ch * 512:(ch + 1) * 512], pq[ch][:],
                        mybir.ActivationFunctionType.Identity,
                        bias=bias[:])

        def score_steps(h):
            # one closure per (j, p_i) score tile. Diagonal blocks get
            # halved exp/mask work:
            #   j == 2p_i:   only the left 128 q-cols need masking
            #   j == 2p_i+1: left 128 q-cols are fully masked -> store a
            #                [P, P] tile of the right half only
            qT, kT = qTs[h], kTs[h]
            steps = []
            for j in range(2 * NQP):
                for p_i in range(j // 2, NQP):
                    def step(j=j, p_i=p_i):
                        qs = slice(p_i * QP, (p_i + 1) * QP)
                        sT = psB.tile([P, QP], F32, tag="sT", bufs=2,
                                      name=f"sT{h}_{j}_{p_i}")
                        nc.tensor.matmul(
                            sT[:], kT[:, j * P:(j + 1) * P], qT[:, qs],
                            start=True, stop=True)
                        if j == 2 * p_i + 1:
                            ex = exps_pool.tile([P, P], a8_dt, tag="exps_h",
                                                name=f"ex{h}_{j}_{p_i}")
                            nc.scalar.activation(
                                ex[:], sT[:, P:QP],
                                mybir.ActivationFunctionType.Exp,
                                scale=SM_SCALE)
                            with nc.allow_low_precision(
                                    reason="exact 0/1 mask"):
                                nc.vector.tensor_mul(ex[:], ex[:], maskT[:])
                        else:
                            ex = exps_pool.tile([P, QP], a8_dt, tag="exps",
                                                name=f"ex{h}_{j}_{p_i}")
                            nc.scalar.activation(
                                ex[:], sT[:],
                                mybir.ActivationFunctionType.Exp,
                                scale=SM_SCALE)
                            if j == 2 * p_i:
                                with nc.allow_low_precision(
                                        reason="exact 0/1 mask"):
                                    nc.vector.tensor_mul(
                                        ex[:, 0:P], ex[:, 0:P], maskT[:])
                        exps[(h, j, p_i)] = ex
                    steps.append(step)
            return steps

        y_norms = {}

        def av_steps(h):
            # one closure per q-block: accumulate yacc2[q, 0:128] = raw y,
            # [:, 128] = softmax denom, then normalize on DVE.
            steps = []
            for qb in range(TT):
                def step(qb=qb):
                    p_i, half = qb // 2, (qb % 2) * P
                    yacc2 = psB.tile([P, P + 1], F32, tag="yacc2", bufs=2,
                                     name=f"yacc2_{h}_{qb}")
                    for j in range(qb + 1):
                        ex = exps[(h, j, p_i)]
                        exs = ex[:, 0:P] if ex.shape[1] == P \
                            else ex[:, half:half + P]
                        nc.tensor.matmul(
                            yacc2[:], exs,
                            vh[h][:, j * VW: j * VW + P + 1],
                            start=(j == 0), stop=(j == qb))
                    rden = att.tile([P, 1], F32, tag="rden", bufs=4,
                                    name=f"rden{h}_{qb}")
                    with nc.allow_low_precision(
                            reason="softmax denom recip is within budget"):
                        nc.vector.reciprocal(rden[:], yacc2[:, P:P + 1])
                    y_norm = att.tile([P, P], at_dt, tag="y_norm", bufs=8,
                                      name=f"yn{h}_{qb}")
                    with nc.allow_low_precision(
                            reason="bf16 y is within budget"):
                        nc.vector.tensor_scalar_mul(
                            y_norm[:], yacc2[:, 0:P], rden[:])
                    y_norms[(h, qb)] = y_norm
                steps.append(step)
            return steps

        def emit_ytp(h):
            # PE transposes of the normalized y blocks -> resident yTh
            for qb in range(TT):
                ytp = psB.tile([P, P], at_dt, tag="ytp", bufs=2,
                               name=f"ytp{h}_{qb}")
                nc.tensor.transpose(ytp[:], y_norms[(h, qb)][:], ident_b[:])
                nc.scalar.activation(
                    yTh[h][:, qb * P:(qb + 1) * P], ytp[:],
                    mybir.ActivationFunctionType.Copy)
                del y_norms[(h, qb)]
            for key in [k for k in exps if k[0] == h]:
                del exps[key]

        def interleave(sc, av):
            # spread the (few, chain-heavy) av groups between the (many)
            # score tiles so the PE always has independent matmuls while
            # the ACT engine drains the exp queue
            out, ai = [], 0
            for si, s_step in enumerate(sc):
                out.append(s_step)
                want = (si + 1) * len(av) // len(sc)
                while ai < want:
                    out.append(av[ai])
                    ai += 1
            out.extend(av[ai:])
            return out

        for h in range(NH):
            if h == NH - 1:
                # prefetch the first W_proj tiles while attention drains
                for hh in range(N_PRE):
                    nc.sync.dma_start(wp_pre[hh][:], r(wp_rows[hh]))
            emit_qk(h)
            sc = score_steps(h)
            av = av_steps(h - 1) if h > 0 else []
            for step in interleave(sc, av):
                step()
            if h > 0:
                emit_ytp(h - 1)
        for step in av_steps(NH - 1):
            step()
        emit_ytp(NH - 1)
        work_ctx.close()

        # ---------------- phase D: out = y @ W_proj + b -------------------
        # hh-outer inside 512-wide n-chunks; 8 PSUM banks hold [T, 512].
        with tc.tile_pool(name="psC", bufs=1, space="PSUM") as psC, \
             tc.tile_pool(name="ph3", bufs=1) as ph3:
            wp = list(wp_pre)
            for hh in range(len(wp_pre), NH):
                wpc = ph3.tile([P, C], mm_dt, tag=f"wp{hh}", bufs=1,
                               name=f"wp{hh}")
                nc.sync.dma_start(wpc[:], r(wp_rows[hh]))
                wp.append(wpc)
            for nn in range(4):
                ns = slice(nn * 512, (nn + 1) * 512)
                po = [psC.tile([P, 512], F32, tag=f"po{i}", bufs=1,
                               name=f"po{nn}_{i}")
                      for i in range(TT)]
                for hh in range(NH):
                    for i in range(TT):
                        nc.tensor.matmul(
                            po[i][:], yTh[hh][:, i * P:(i + 1) * P],
                            wp[hh][:, ns],
                            start=(hh == 0), stop=False)
                for i in range(TT):
                    nc.tensor.matmul(
                        po[i][:], ones_row[:], bp_sb[:, ns],
                        start=False, stop=True)
                    osb = ph3.tile([P, 512], F32, tag="osb", bufs=2,
                                   name=f"osb{nn}_{i}")
                    if i % 4 == 1:
                        nc.vector.tensor_copy(osb[:], po[i][:])
                    elif i % 4 == 3:
                        nc.gpsimd.tensor_copy(osb[:], po[i][:])
                    else:
                        nc.scalar.activation(
                            osb[:], po[i][:],
                            mybir.ActivationFunctionType.Copy)
                    nc.sync.dma_start(
                        out_rows[i][:, ns], osb[:])

    return nc


_BUILT = {}


def _get_nc(mode: str):
    if mode not in _BUILT:
        _BUILT[mode] = build_kernel(mode)
    return _BUILT[mode]


def kernel(x, W_attn, b_attn, W_proj, b_proj, mode: str = "f32r", **run_kwargs):
    from concourse.bass_utils import run_bass_kernel_spmd

    x = np.asarray(x, dtype=np.float32)
    W_attn = np.ascontiguousarray(np.asarray(W_attn, dtype=np.float32))
    b_attn = np.ascontiguousarray(np.asarray(b_attn, dtype=np.float32))
    W_proj = np.ascontiguousarray(np.asarray(W_proj, dtype=np.float32))
    b_proj = np.ascontiguousarray(np.asarray(b_proj, dtype=np.float32))

    nc = _get_nc(mode)
    in_maps = [
        {
            "x": np.ascontiguousarray(x[b]),
            "W_attn": W_attn,
            "b_attn": b_attn,
            "W_proj": W_proj,
            "b_proj": b_proj,
        }
        for b in range(N_CORES)
    ]
    res = run_bass_kernel_spmd(nc, in_maps, list(range(N_CORES)), **run_kwargs)
    out = np.stack([res.results[b]["out"] for b in range(N_CORES)], axis=0)
    kernel.last_results = res
    return out
